# revision 3
# baseline (speedup 1.0000x reference)
"""MultiHeadAttention (B=4, S=2048, D=2048, H=16) on 8 TRN2 NeuronCores.

Sharding: core c handles batch b = c//2 and head-half = c%2 (8 heads).
Each core computes Q/K/V projections for its 1024 rows, attention for its
8 heads, and a partial output projection; the host sums the two partials
per batch and un-permutes.

Layout trick: torch's `view(B, H, S, dk)` head split (no transpose) means
head h of batch b lives in rows [128h, 128h+128) of the projection output,
with each row holding 16 consecutive seq positions. Working in permuted
query/key coordinates pi = 128*t + u (s = 16*u + t), every attention
operand is an exact 128x128 tile of either the transposed projection
(R^T, for Q/K) or the natural projection (R, for V). Softmax is
permutation-invariant, and the host un-permutes the final output.

All matmuls run in float32r (fp32 with 10-bit mantissa, full PE speed);
host pre-rounds all external matmul operands.
"""
import math
import os

import numpy as np

B, S, D, H = 4, 2048, 2048, 16
DK = D // H            # 128
HPC = H // 2           # heads per core = 8
RPC = HPC * DK         # rows per core = 1024
NC_ = 8                # cores
MC = D // 128          # contraction chunks = 16
SCALE = 1.0 / math.sqrt(DK)

_cache = {}
last_results = None


def _round_f32r(x):
    """Round fp32 to the 10-bit-mantissa grid the PE uses for float32r."""
    x = np.ascontiguousarray(x, dtype=np.float32)
    u = x.view(np.uint32)
    lsb = (u >> np.uint32(13)) & np.uint32(1)
    r = (u + np.uint32(0x0FFF) + lsb) & np.uint32(0xFFFFE000)
    return r.view(np.float32)


def _build():
    import concourse.bass as bass
    import concourse.mybir as mybir
    import concourse.tile as tile
    from concourse import bacc

    f32 = mybir.dt.float32
    f32r = mybir.dt.float32r
    AF = mybir.ActivationFunctionType

    nc = bacc.Bacc("TRN2", target_bir_lowering=False, debug=False,
                   num_devices=NC_)

    # ---- external I/O ----
    qts_d = nc.dram_tensor("qts", (D, RPC), f32r, kind="ExternalInput")
    kts_d = nc.dram_tensor("kts", (D, RPC), f32r, kind="ExternalInput")
    vts_d = nc.dram_tensor("vts", (D, RPC), f32r, kind="ExternalInput")
    wqt_d = nc.dram_tensor("wqt", (D, D), f32r, kind="ExternalInput")
    wkt_d = nc.dram_tensor("wkt", (D, D), f32r, kind="ExternalInput")
    wvt_d = nc.dram_tensor("wvt", (D, D), f32r, kind="ExternalInput")
    wot_d = nc.dram_tensor("wot", (RPC, D), f32r, kind="ExternalInput")
    bqs_d = nc.dram_tensor("bqs", (D,), f32, kind="ExternalInput")
    bk_d = nc.dram_tensor("bk", (D,), f32, kind="ExternalInput")
    bvr_d = nc.dram_tensor("bvr", (1, D), f32r, kind="ExternalInput")
    bo_d = nc.dram_tensor("bo", (D,), f32, kind="ExternalInput")
    ones1_d = nc.dram_tensor("ones1", (1, 128), f32r, kind="ExternalInput")
    onescol_d = nc.dram_tensor("onescol", (128, 1), f32r, kind="ExternalInput")
    out_d = nc.dram_tensor("out", (D, S), f32, kind="ExternalOutput")

    with tile.TileContext(nc) as tc:
        with (
            tc.tile_pool(name="resident", bufs=1) as rpool,
            tc.tile_pool(name="dram", bufs=1, space="DRAM") as dpool,
        ):
            # long-lived SBUF
            kt_sb = rpool.tile([128, MC, RPC], f32r)      # R_k^T  [64KB/part]
            bq_sb = rpool.tile([128, MC], f32)
            bk_sb = rpool.tile([128, MC], f32)
            bo_sb = rpool.tile([128, MC], f32)
            bv_sb = rpool.tile([1, D], f32r)
            ones1 = rpool.tile([1, 128], f32r)
            onescol = rpool.tile([128, 1], f32r)
            nc.sync.dma_start(bq_sb[:], bqs_d.ap().rearrange("(t p) -> p t", p=128))
            nc.sync.dma_start(bk_sb[:], bk_d.ap().rearrange("(t p) -> p t", p=128))
            nc.sync.dma_start(bo_sb[:], bo_d.ap().rearrange("(t p) -> p t", p=128))
            nc.sync.dma_start(bv_sb[:], bvr_d.ap())
            nc.sync.dma_start(ones1[:], ones1_d.ap())
            nc.sync.dma_start(onescol[:], onescol_d.ap())

            # DRAM scratch
            qhat_dram = dpool.tile([D, RPC], f32r)    # rows = 128*t + dk
            vhat_dram = dpool.tile([RPC, D], f32r)    # natural R_v

            # ---------------- phase V: R_v = VTs^T @ WvT + bv -------------
            with (
                nc.named_scope("proj_v"),
                tc.tile_pool(name="vin", bufs=1) as vin_pool,
                tc.tile_pool(name="wv", bufs=2) as wv_pool,
                tc.tile_pool(name="vps", bufs=4, space="PSUM") as vps_pool,
                tc.tile_pool(name="vout", bufs=4) as vout_pool,
            ):
                vt_st = vin_pool.tile([128, MC, RPC], f32r)
                for mc in range(MC):
                    nc.sync.dma_start(
                        vt_st[:, mc, :], vts_d.ap()[128 * mc:128 * mc + 128, :])
                for cb in range(8):           # c blocks of 256
                    wv_st = wv_pool.tile([128, MC, 256], f32r, tag="wv")
                    for mc in range(MC):
                        nc.sync.dma_start(
                            wv_st[:, mc, :],
                            wvt_d.ap()[128 * mc:128 * mc + 128,
                                       256 * cb:256 * cb + 256])
                    for rt in range(8):       # r tiles of 128
                        ps = vps_pool.tile([128, 256], f32, tag="vps")
                        for mc in range(MC):
                            nc.tensor.matmul(
                                ps[:], vt_st[:, mc, 128 * rt:128 * rt + 128],
                                wv_st[:, mc, :], start=(mc == 0), stop=False)
                        nc.tensor.matmul(
                            ps[:], ones1[:],
                            bv_sb[:, 256 * cb:256 * cb + 256],
                            start=False, stop=True)
                        vo = vout_pool.tile([128, 256], f32r, tag="vo")
                        nc.vector.tensor_copy(vo[:], ps[:])
                        nc.sync.dma_start(
                            vhat_dram[128 * rt:128 * rt + 128,
                                      256 * cb:256 * cb + 256], vo[:])

            # ---------------- phase Q: R_q^T = WqT^T @ QTs (scaled) -------
            with (
                nc.named_scope("proj_q"),
                tc.tile_pool(name="qin", bufs=1) as qin_pool,
                tc.tile_pool(name="wq", bufs=3) as wq_pool,
                tc.tile_pool(name="qps", bufs=4, space="PSUM") as qps_pool,
                tc.tile_pool(name="qout", bufs=4) as qout_pool,
            ):
                qt_st = qin_pool.tile([128, MC, RPC], f32r)
                for mc in range(MC):
                    nc.sync.dma_start(
                        qt_st[:, mc, :], qts_d.ap()[128 * mc:128 * mc + 128, :])
                for ct in range(MC):
                    wq_st = wq_pool.tile([128, MC, 128], f32r, tag="wq")
                    for mc in range(MC):
                        nc.sync.dma_start(
                            wq_st[:, mc, :],
                            wqt_d.ap()[128 * mc:128 * mc + 128,
                                       128 * ct:128 * ct + 128])
                    for rb in range(2):
                        ps = qps_pool.tile([128, 512], f32, tag="qps")
                        for mc in range(MC):
                            nc.tensor.matmul(
                                ps[:], wq_st[:, mc, :],
                                qt_st[:, mc, 512 * rb:512 * rb + 512],
                                start=(mc == 0), stop=(mc == MC - 1))
                        qo = qout_pool.tile([128, 512], f32r, tag="qo")
                        nc.scalar.activation(qo[:], ps[:], AF.Identity,
                                             bias=bq_sb[:, ct:ct + 1],
                                             scale=SCALE)
                        nc.sync.dma_start(
                            qhat_dram[128 * ct:128 * ct + 128,
                                      512 * rb:512 * rb + 512], qo[:])

            # ---------------- phase K: R_k^T -> SBUF ----------------------
            with (
                nc.named_scope("proj_k"),
                tc.tile_pool(name="kin", bufs=1) as kin_pool,
                tc.tile_pool(name="wk", bufs=3) as wk_pool,
                tc.tile_pool(name="kps", bufs=4, space="PSUM") as kps_pool,
            ):
                kt_st = kin_pool.tile([128, MC, RPC], f32r)
                for mc in range(MC):
                    nc.sync.dma_start(
                        kt_st[:, mc, :], kts_d.ap()[128 * mc:128 * mc + 128, :])
                for ct in range(MC):
                    wk_st = wk_pool.tile([128, MC, 128], f32r, tag="wk")
                    for mc in range(MC):
                        nc.sync.dma_start(
                            wk_st[:, mc, :],
                            wkt_d.ap()[128 * mc:128 * mc + 128,
                                       128 * ct:128 * ct + 128])
                    for rb in range(2):
                        ps = kps_pool.tile([128, 512], f32, tag="kps")
                        for mc in range(MC):
                            nc.tensor.matmul(
                                ps[:], wk_st[:, mc, :],
                                kt_st[:, mc, 512 * rb:512 * rb + 512],
                                start=(mc == 0), stop=(mc == MC - 1))
                        nc.scalar.activation(
                            kt_sb[:, ct, 512 * rb:512 * rb + 512], ps[:],
                            AF.Identity, bias=bk_sb[:, ct:ct + 1], scale=1.0)

            # ---------------- attention + output projection ---------------
            qhat_v = qhat_dram[:].rearrange("(t p) r -> p t r", p=128)
            with (
                nc.named_scope("attn"),
                tc.tile_pool(name="qrhs", bufs=3) as q_pool,
                tc.tile_pool(name="vh", bufs=2) as vh_pool,
                tc.tile_pool(name="expp", bufs=18) as exp_pool,
                tc.tile_pool(name="scps", bufs=3, space="PSUM") as scps_pool,
                tc.tile_pool(name="xps", bufs=2, space="PSUM") as xps_pool,
                tc.tile_pool(name="sps", bufs=2, space="PSUM") as sps_pool,
                tc.tile_pool(name="ops", bufs=1, space="PSUM") as ops_pool,
                tc.tile_pool(name="nrm", bufs=2) as nrm_pool,
                tc.tile_pool(name="xsb", bufs=2) as x_pool,
                tc.tile_pool(name="wo", bufs=3) as wo_pool,
                tc.tile_pool(name="oout", bufs=4) as oout_pool,
            ):
                for j in range(4):            # query pi-blocks of 512
                    x_j = x_pool.tile([128, HPC, 512], f32r, tag="xj")
                    for h in range(HPC):
                        q_rhs = q_pool.tile([128, 4, 128], f32r, tag="qr")
                        for t in range(4):
                            nc.sync.dma_start(
                                q_rhs[:, t, :],
                                qhat_v[:, 4 * j + t, 128 * h:128 * h + 128])
                        v_h = vh_pool.tile([128, MC, 128], f32r, tag="vh")
                        for tk in range(MC):
                            nc.sync.dma_start(
                                v_h[:, tk, :],
                                vhat_dram[128 * h:128 * h + 128,
                                          128 * tk:128 * tk + 128])
                        x_ps = xps_pool.tile([128, 512], f32, tag="xps")
                        s_ps = sps_pool.tile([1, 512], f32, tag="sps")
                        for tk in range(MC):
                            sc = scps_pool.tile([128, 512], f32, tag="sc")
                            nc.tensor.matmul(
                                sc[:], kt_sb[:, tk, 128 * h:128 * h + 128],
                                q_rhs[:, :, :], start=True, stop=True)
                            ex = exp_pool.tile([128, 512], f32r, tag="ex")
                            nc.scalar.activation(ex[:], sc[:], AF.Exp,
                                                 scale=1.0)
                            nc.tensor.matmul(x_ps[:], v_h[:, tk, :], ex[:],
                                             start=(tk == 0),
                                             stop=(tk == MC - 1))
                            nc.tensor.matmul(s_ps[:], onescol[:], ex[:],
                                             start=(tk == 0),
                                             stop=(tk == MC - 1))
                        rec = nrm_pool.tile([1, 512], f32, tag="rec")
                        nc.vector.reciprocal(rec[:], s_ps[:])
                        bcast = nrm_pool.tile([128, 512], f32, tag="bc")
                        nc.gpsimd.partition_broadcast(bcast[:], rec[:])
                        nc.vector.tensor_mul(x_j[:, h, :], x_ps[:], bcast[:])
                    # output projection for this pi-block
                    for ot in range(MC):
                        wo_st = wo_pool.tile([128, HPC, 128], f32r, tag="wo")
                        for h in range(HPC):
                            nc.sync.dma_start(
                                wo_st[:, h, :],
                                wot_d.ap()[128 * h:128 * h + 128,
                                           128 * ot:128 * ot + 128])
                        op = ops_pool.tile([128, 512], f32, tag="op")
                        for h in range(HPC):
                            nc.tensor.matmul(op[:], wo_st[:, h, :],
                                             x_j[:, h, :], start=(h == 0),
                                             stop=(h == HPC - 1))
                        oo = oout_pool.tile([128, 512], f32, tag="oo")
                        nc.scalar.activation(oo[:], op[:], AF.Identity,
                                             bias=bo_sb[:, ot:ot + 1],
                                             scale=1.0)
                        nc.sync.dma_start(
                            out_d.ap()[128 * ot:128 * ot + 128,
                                       512 * j:512 * j + 512], oo[:])

    nc.compile()
    return nc


def kernel(Q, K, V, Wq, bq, Wk, bk, Wv, bv, Wo, bo, num_heads):
    global last_results
    assert int(num_heads) == H

    from concourse.bass_utils import run_bass_kernel_spmd

    if "nc" not in _cache:
        _cache["nc"] = _build()
    nc = _cache["nc"]

    Q = np.asarray(Q, np.float32)
    K = np.asarray(K, np.float32)
    V = np.asarray(V, np.float32)
    wqt = _round_f32r(np.ascontiguousarray(np.asarray(Wq, np.float32).T))
    wkt = _round_f32r(np.ascontiguousarray(np.asarray(Wk, np.float32).T))
    wvt = _round_f32r(np.ascontiguousarray(np.asarray(Wv, np.float32).T))
    woT = np.ascontiguousarray(np.asarray(Wo, np.float32).T)
    bqs = (np.asarray(bq, np.float32) * SCALE).copy()
    bk_np = np.asarray(bk, np.float32).copy()
    bvr = _round_f32r(np.asarray(bv, np.float32).reshape(1, D))
    bo_np = np.asarray(bo, np.float32).copy()
    ones1 = np.ones((1, 128), np.float32)
    onescol = np.ones((128, 1), np.float32)

    in_maps = []
    for c in range(NC_):
        b, half = divmod(c, 2)
        r0 = RPC * half
        in_maps.append({
            "qts": _round_f32r(Q[b].T[:, r0:r0 + RPC]),
            "kts": _round_f32r(K[b].T[:, r0:r0 + RPC]),
            "vts": _round_f32r(V[b].T[:, r0:r0 + RPC]),
            "wqt": wqt, "wkt": wkt, "wvt": wvt,
            "wot": _round_f32r(woT[r0:r0 + RPC, :]),
            "bqs": bqs, "bk": bk_np, "bvr": bvr, "bo": bo_np,
            "ones1": ones1, "onescol": onescol,
        })

    res = run_bass_kernel_spmd(nc, in_maps, core_ids=list(range(NC_)))
    last_results = res

    out = np.empty((B, S, D), np.float32)
    for b in range(B):
        oT = res.results[2 * b]["out"] + res.results[2 * b + 1]["out"]
        # oT[o, pi], pi = 128*t + u ; s = 16*u + t
        out[b] = oT.reshape(D, 16, 128).transpose(2, 1, 0).reshape(S, D)
    return out


# revision 4
# speedup vs baseline: 1.2543x; 1.2543x over previous
"""MultiHeadAttention (B=4, S=2048, D=2048, H=16) on 8 TRN2 NeuronCores.

Sharding: core c handles batch b = c//2 and head-half = c%2 (8 heads).
Each core computes Q/K/V projections for its 1024 rows, attention for its
8 heads, and a partial output projection; the host sums the two partials
per batch and un-permutes.

Layout trick: torch's `view(B, H, S, dk)` head split (no transpose) means
head h of batch b lives in rows [128h, 128h+128) of the projection output,
with each row holding 16 consecutive seq positions. Working in permuted
query/key coordinates pi = 128*t + u (s = 16*u + t), every attention
operand is an exact 128x128 tile of either the transposed projection
(R^T, for Q/K) or the natural projection (R, for V). Softmax is
permutation-invariant, and the host un-permutes the final output.

All matmuls run in float32r (fp32 with 10-bit mantissa, full PE speed);
host pre-rounds all external matmul operands. Weights are pre-tiled on
the host so every SBUF stage tile loads as one DMA with >=4KB contiguous
runs per partition.
"""
import math
import os

import numpy as np

B, S, D, H = 4, 2048, 2048, 16
DK = D // H            # 128
HPC = H // 2           # heads per core = 8
RPC = HPC * DK         # rows per core = 1024
NC_ = 8                # cores
MC = D // 128          # contraction chunks = 16
SCALE = 1.0 / math.sqrt(DK)

_cache = {}
last_results = None


def _round_f32r(x):
    """Round fp32 to the 10-bit-mantissa grid the PE uses for float32r."""
    x = np.ascontiguousarray(x, dtype=np.float32)
    u = x.view(np.uint32)
    lsb = (u >> np.uint32(13)) & np.uint32(1)
    r = (u + np.uint32(0x0FFF) + lsb) & np.uint32(0xFFFFE000)
    return r.view(np.float32)


def _build():
    import concourse.bass as bass
    import concourse.mybir as mybir
    import concourse.tile as tile
    from concourse import bacc

    f32 = mybir.dt.float32
    f32r = mybir.dt.float32r
    AF = mybir.ActivationFunctionType

    nc = bacc.Bacc("TRN2", target_bir_lowering=False, debug=False,
                   num_devices=NC_)

    # ---- external I/O ----
    # transposed input slices (m-chunked): [mc][p][r]
    qts_d = nc.dram_tensor("qts", (MC, 128, RPC), f32r, kind="ExternalInput")
    kts_d = nc.dram_tensor("kts", (MC, 128, RPC), f32r, kind="ExternalInput")
    vts_d = nc.dram_tensor("vts", (MC, 128, RPC), f32r, kind="ExternalInput")
    # pre-tiled weights: wq/wk [ct][p][mc][c], wv [cb][p][mc][c256]
    wqt_d = nc.dram_tensor("wqt", (MC, 128, MC, 128), f32r, kind="ExternalInput")
    wkt_d = nc.dram_tensor("wkt", (MC, 128, MC, 128), f32r, kind="ExternalInput")
    wvt_d = nc.dram_tensor("wvt", (8, 128, MC, 256), f32r, kind="ExternalInput")
    # pre-tiled output weights: [ot][dk][h][o]
    wot_d = nc.dram_tensor("wot", (MC, 128, HPC, 128), f32r, kind="ExternalInput")
    bqs_d = nc.dram_tensor("bqs", (D,), f32, kind="ExternalInput")
    bk_d = nc.dram_tensor("bk", (D,), f32, kind="ExternalInput")
    bvr_d = nc.dram_tensor("bvr", (1, D), f32r, kind="ExternalInput")
    bo_d = nc.dram_tensor("bo", (D,), f32, kind="ExternalInput")
    ones1_d = nc.dram_tensor("ones1", (1, 128), f32r, kind="ExternalInput")
    onescol_d = nc.dram_tensor("onescol", (128, 1), f32r, kind="ExternalInput")
    out_d = nc.dram_tensor("out", (D, S), f32, kind="ExternalOutput")

    with tile.TileContext(nc) as tc:
        with (
            tc.tile_pool(name="resident", bufs=1) as rpool,
            tc.tile_pool(name="dram", bufs=1, space="DRAM") as dpool,
        ):
            # long-lived SBUF
            kt_sb = rpool.tile([128, MC, RPC], f32r)      # R_k^T  [64KB/part]
            bq_sb = rpool.tile([128, MC], f32)
            bk_sb = rpool.tile([128, MC], f32)
            bo_sb = rpool.tile([128, MC], f32)
            bv_sb = rpool.tile([1, D], f32r)
            ones1 = rpool.tile([1, 128], f32r)
            onescol = rpool.tile([128, 1], f32r)
            nc.sync.dma_start(bq_sb[:], bqs_d.ap().rearrange("(t p) -> p t", p=128))
            nc.sync.dma_start(bk_sb[:], bk_d.ap().rearrange("(t p) -> p t", p=128))
            nc.sync.dma_start(bo_sb[:], bo_d.ap().rearrange("(t p) -> p t", p=128))
            nc.sync.dma_start(bv_sb[:], bvr_d.ap())
            nc.sync.dma_start(ones1[:], ones1_d.ap())
            nc.sync.dma_start(onescol[:], onescol_d.ap())

            # DRAM scratch
            qhat_dram = dpool.tile([MC, 128, RPC], f32r)  # [ct][dk][r]
            vhat_dram = dpool.tile([RPC, D], f32r)        # natural R_v

            # ---------------- phase V: R_v = VTs^T @ WvT + bv -------------
            with (
                nc.named_scope("proj_v"),
                tc.tile_pool(name="vin", bufs=1) as vin_pool,
                tc.tile_pool(name="wv", bufs=2) as wv_pool,
                tc.tile_pool(name="vps", bufs=4, space="PSUM") as vps_pool,
                tc.tile_pool(name="vout", bufs=4) as vout_pool,
            ):
                vt_st = vin_pool.tile([128, MC, RPC], f32r)
                for mc in range(MC):
                    nc.sync.dma_start(vt_st[:, mc, :], vts_d.ap()[mc])
                for cb in range(8):           # c blocks of 256
                    wv_st = wv_pool.tile([128, MC, 256], f32r, tag="wv")
                    nc.sync.dma_start(wv_st[:], wvt_d.ap()[cb])
                    for rt in range(8):       # r tiles of 128
                        ps = vps_pool.tile([128, 256], f32, tag="vps")
                        for mc in range(MC):
                            nc.tensor.matmul(
                                ps[:], vt_st[:, mc, 128 * rt:128 * rt + 128],
                                wv_st[:, mc, :], start=(mc == 0), stop=False)
                        nc.tensor.matmul(
                            ps[:], ones1[:],
                            bv_sb[:, 256 * cb:256 * cb + 256],
                            start=False, stop=True)
                        vo = vout_pool.tile([128, 256], f32r, tag="vo")
                        nc.vector.tensor_copy(vo[:], ps[:])
                        nc.gpsimd.dma_start(
                            vhat_dram[128 * rt:128 * rt + 128,
                                      256 * cb:256 * cb + 256], vo[:])

            # ---------------- phase Q: R_q^T = WqT^T @ QTs (scaled) -------
            with (
                nc.named_scope("proj_q"),
                tc.tile_pool(name="qin", bufs=1) as qin_pool,
                tc.tile_pool(name="wq", bufs=3) as wq_pool,
                tc.tile_pool(name="qps", bufs=4, space="PSUM") as qps_pool,
                tc.tile_pool(name="qout", bufs=4) as qout_pool,
            ):
                qt_st = qin_pool.tile([128, MC, RPC], f32r)
                for mc in range(MC):
                    nc.sync.dma_start(qt_st[:, mc, :], qts_d.ap()[mc])
                for ct in range(MC):
                    wq_st = wq_pool.tile([128, MC, 128], f32r, tag="wq")
                    nc.sync.dma_start(wq_st[:], wqt_d.ap()[ct])
                    for rb in range(2):
                        ps = qps_pool.tile([128, 512], f32, tag="qps")
                        for mc in range(MC):
                            nc.tensor.matmul(
                                ps[:], wq_st[:, mc, :],
                                qt_st[:, mc, 512 * rb:512 * rb + 512],
                                start=(mc == 0), stop=(mc == MC - 1))
                        qo = qout_pool.tile([128, 512], f32r, tag="qo")
                        nc.scalar.activation(qo[:], ps[:], AF.Identity,
                                             bias=bq_sb[:, ct:ct + 1],
                                             scale=SCALE)
                        nc.gpsimd.dma_start(
                            qhat_dram[ct, :, 512 * rb:512 * rb + 512], qo[:])

            # ---------------- phase K: R_k^T -> SBUF ----------------------
            with (
                nc.named_scope("proj_k"),
                tc.tile_pool(name="kin", bufs=1) as kin_pool,
                tc.tile_pool(name="wk", bufs=3) as wk_pool,
                tc.tile_pool(name="kps", bufs=4, space="PSUM") as kps_pool,
            ):
                kt_st = kin_pool.tile([128, MC, RPC], f32r)
                for mc in range(MC):
                    nc.sync.dma_start(kt_st[:, mc, :], kts_d.ap()[mc])
                for ct in range(MC):
                    wk_st = wk_pool.tile([128, MC, 128], f32r, tag="wk")
                    nc.sync.dma_start(wk_st[:], wkt_d.ap()[ct])
                    for rb in range(2):
                        ps = kps_pool.tile([128, 512], f32, tag="kps")
                        for mc in range(MC):
                            nc.tensor.matmul(
                                ps[:], wk_st[:, mc, :],
                                kt_st[:, mc, 512 * rb:512 * rb + 512],
                                start=(mc == 0), stop=(mc == MC - 1))
                        nc.scalar.activation(
                            kt_sb[:, ct, 512 * rb:512 * rb + 512], ps[:],
                            AF.Identity, bias=bk_sb[:, ct:ct + 1], scale=1.0)

            # ---------------- attention + output projection ---------------
            with (
                nc.named_scope("attn"),
                tc.tile_pool(name="qrhs", bufs=3) as q_pool,
                tc.tile_pool(name="vh", bufs=2) as vh_pool,
                tc.tile_pool(name="expp", bufs=18) as exp_pool,
                tc.tile_pool(name="scps", bufs=3, space="PSUM") as scps_pool,
                tc.tile_pool(name="xps", bufs=2, space="PSUM") as xps_pool,
                tc.tile_pool(name="sps", bufs=2, space="PSUM") as sps_pool,
                tc.tile_pool(name="ops", bufs=1, space="PSUM") as ops_pool,
                tc.tile_pool(name="nrm", bufs=2) as nrm_pool,
                tc.tile_pool(name="xsb", bufs=2) as x_pool,
                tc.tile_pool(name="wo", bufs=3) as wo_pool,
                tc.tile_pool(name="oout", bufs=4) as oout_pool,
            ):
                for j in range(4):            # query pi-blocks of 512
                    x_j = x_pool.tile([128, HPC, 512], f32r, tag="xj")
                    for h in range(HPC):
                        q_rhs = q_pool.tile([128, 4, 128], f32r, tag="qr")
                        for t in range(4):
                            nc.sync.dma_start(
                                q_rhs[:, t, :],
                                qhat_dram[4 * j + t, :,
                                          128 * h:128 * h + 128])
                        v_h = vh_pool.tile([128, MC, 128], f32r, tag="vh")
                        nc.sync.dma_start(
                            v_h[:], vhat_dram[128 * h:128 * h + 128, :])
                        x_ps = xps_pool.tile([128, 512], f32, tag="xps")
                        s_ps = sps_pool.tile([1, 512], f32, tag="sps")
                        for tk in range(MC):
                            sc = scps_pool.tile([128, 512], f32, tag="sc")
                            nc.tensor.matmul(
                                sc[:], kt_sb[:, tk, 128 * h:128 * h + 128],
                                q_rhs[:, :, :], start=True, stop=True)
                            ex = exp_pool.tile([128, 512], f32r, tag="ex")
                            nc.scalar.activation(ex[:], sc[:], AF.Exp,
                                                 scale=1.0)
                            nc.tensor.matmul(x_ps[:], v_h[:, tk, :], ex[:],
                                             start=(tk == 0),
                                             stop=(tk == MC - 1))
                            nc.tensor.matmul(s_ps[:], onescol[:], ex[:],
                                             start=(tk == 0),
                                             stop=(tk == MC - 1))
                        rec = nrm_pool.tile([1, 512], f32, tag="rec")
                        nc.vector.reciprocal(rec[:], s_ps[:])
                        bcast = nrm_pool.tile([128, 512], f32, tag="bc")
                        nc.gpsimd.partition_broadcast(bcast[:], rec[:])
                        nc.vector.tensor_mul(x_j[:, h, :], x_ps[:], bcast[:])
                    # output projection for this pi-block
                    for ot in range(MC):
                        wo_st = wo_pool.tile([128, HPC, 128], f32r, tag="wo")
                        nc.sync.dma_start(wo_st[:], wot_d.ap()[ot])
                        op = ops_pool.tile([128, 512], f32, tag="op")
                        for h in range(HPC):
                            nc.tensor.matmul(op[:], wo_st[:, h, :],
                                             x_j[:, h, :], start=(h == 0),
                                             stop=(h == HPC - 1))
                        oo = oout_pool.tile([128, 512], f32, tag="oo")
                        nc.scalar.activation(oo[:], op[:], AF.Identity,
                                             bias=bo_sb[:, ot:ot + 1],
                                             scale=1.0)
                        nc.gpsimd.dma_start(
                            out_d.ap()[128 * ot:128 * ot + 128,
                                       512 * j:512 * j + 512], oo[:])

    nc.compile()
    return nc


def _prep_shared(Wq, Wk, Wv, Wo, bq, bk, bv, bo):
    wqt = _round_f32r(np.ascontiguousarray(np.asarray(Wq, np.float32).T))
    wkt = _round_f32r(np.ascontiguousarray(np.asarray(Wk, np.float32).T))
    wvt = _round_f32r(np.ascontiguousarray(np.asarray(Wv, np.float32).T))
    # wq/wk tiled: [ct][p=m%128][mc][c]
    wqt_t = np.ascontiguousarray(
        wqt.reshape(MC, 128, MC, 128).transpose(2, 1, 0, 3))
    wkt_t = np.ascontiguousarray(
        wkt.reshape(MC, 128, MC, 128).transpose(2, 1, 0, 3))
    # wv tiled: [cb][p][mc][c256]
    wvt_t = np.ascontiguousarray(
        wvt.reshape(MC, 128, 8, 256).transpose(2, 1, 0, 3))
    woT = np.ascontiguousarray(np.asarray(Wo, np.float32).T)
    bqs = (np.asarray(bq, np.float32) * SCALE).copy()
    bk_np = np.asarray(bk, np.float32).copy()
    bvr = _round_f32r(np.asarray(bv, np.float32).reshape(1, D))
    bo_np = np.asarray(bo, np.float32).copy()
    return wqt_t, wkt_t, wvt_t, woT, bqs, bk_np, bvr, bo_np


def kernel(Q, K, V, Wq, bq, Wk, bk, Wv, bv, Wo, bo, num_heads):
    global last_results
    assert int(num_heads) == H

    from concourse.bass_utils import run_bass_kernel_spmd

    if "nc" not in _cache:
        _cache["nc"] = _build()
    nc = _cache["nc"]

    Q = np.asarray(Q, np.float32)
    K = np.asarray(K, np.float32)
    V = np.asarray(V, np.float32)
    wqt_t, wkt_t, wvt_t, woT, bqs, bk_np, bvr, bo_np = _prep_shared(
        Wq, Wk, Wv, Wo, bq, bk, bv, bo)
    ones1 = np.ones((1, 128), np.float32)
    onescol = np.ones((128, 1), np.float32)

    in_maps = []
    for c in range(NC_):
        b, half = divmod(c, 2)
        r0 = RPC * half
        # wot tiled: [ot][dk][h][o] from WoT rows r0..r0+1024
        wot_t = np.ascontiguousarray(
            _round_f32r(woT[r0:r0 + RPC, :])
            .reshape(HPC, 128, MC, 128).transpose(2, 1, 0, 3))
        in_maps.append({
            "qts": _round_f32r(Q[b].T[:, r0:r0 + RPC]).reshape(MC, 128, RPC),
            "kts": _round_f32r(K[b].T[:, r0:r0 + RPC]).reshape(MC, 128, RPC),
            "vts": _round_f32r(V[b].T[:, r0:r0 + RPC]).reshape(MC, 128, RPC),
            "wqt": wqt_t, "wkt": wkt_t, "wvt": wvt_t, "wot": wot_t,
            "bqs": bqs, "bk": bk_np, "bvr": bvr, "bo": bo_np,
            "ones1": ones1, "onescol": onescol,
        })

    res = run_bass_kernel_spmd(nc, in_maps, core_ids=list(range(NC_)))
    last_results = res

    out = np.empty((B, S, D), np.float32)
    for b in range(B):
        oT = res.results[2 * b]["out"] + res.results[2 * b + 1]["out"]
        # oT[o, pi], pi = 128*t + u ; s = 16*u + t
        out[b] = oT.reshape(D, 16, 128).transpose(2, 1, 0).reshape(S, D)
    return out


# revision 8
# speedup vs baseline: 1.3385x; 1.0671x over previous
"""MultiHeadAttention (B=4, S=2048, D=2048, H=16) on 8 TRN2 NeuronCores.

Sharding: core c handles batch b = c//2 and head-half = c%2 (8 heads).
Each core computes Q/K/V projections for its 1024 rows, attention for its
8 heads, and a partial output projection; the host sums the two partials
per batch and un-permutes.

Layout trick: torch's `view(B, H, S, dk)` head split (no transpose) means
head h of batch b lives in rows [128h, 128h+128) of the projection output,
with each row holding 16 consecutive seq positions. Working in permuted
query/key coordinates pi = 128*t + u (s = 16*u + t), every attention
operand is an exact 128x128 tile of either the transposed projection
(R^T, for Q/K) or the natural projection (R, for V). Softmax is
permutation-invariant, and the host un-permutes the final output.

All matmuls run in float32r (fp32 with 10-bit mantissa, full PE speed);
host pre-rounds all external matmul operands. Weights are pre-tiled on
the host so every SBUF stage tile loads as one DMA with >=4KB contiguous
runs per partition.
"""
import math
import os

import numpy as np

B, S, D, H = 4, 2048, 2048, 16
DK = D // H            # 128
HPC = H // 2           # heads per core = 8
RPC = HPC * DK         # rows per core = 1024
NC_ = 8                # cores
MC = D // 128          # contraction chunks = 16
SCALE = 1.0 / math.sqrt(DK)

_cache = {}
last_results = None


def _round_f32r(x):
    """Round fp32 to the 10-bit-mantissa grid the PE uses for float32r."""
    x = np.ascontiguousarray(x, dtype=np.float32)
    u = x.view(np.uint32)
    lsb = (u >> np.uint32(13)) & np.uint32(1)
    r = (u + np.uint32(0x0FFF) + lsb) & np.uint32(0xFFFFE000)
    return r.view(np.float32)


def _build():
    import concourse.bass as bass
    import concourse.mybir as mybir
    import concourse.tile as tile
    from concourse import bacc

    f32 = mybir.dt.float32
    f32r = mybir.dt.float32r
    AF = mybir.ActivationFunctionType

    nc = bacc.Bacc("TRN2", target_bir_lowering=False, debug=False,
                   num_devices=NC_)

    # ---- external I/O ----
    # transposed input slices (m-chunked): [mc][p][r]
    qts_d = nc.dram_tensor("qts", (MC, 128, RPC), f32r, kind="ExternalInput")
    kts_d = nc.dram_tensor("kts", (MC, 128, RPC), f32r, kind="ExternalInput")
    vts_d = nc.dram_tensor("vts", (MC, 128, RPC), f32r, kind="ExternalInput")
    # pre-tiled weights: wq/wk [ct][p][mc][c], wv [cb][p][mc][c256]
    wqt_d = nc.dram_tensor("wqt", (MC, 128, MC, 128), f32r, kind="ExternalInput")
    wkt_d = nc.dram_tensor("wkt", (MC, 128, MC, 128), f32r, kind="ExternalInput")
    wvt_d = nc.dram_tensor("wvt", (8, 128, MC, 256), f32r, kind="ExternalInput")
    # pre-tiled output weights: [ot][dk][h][o]
    wot_d = nc.dram_tensor("wot", (MC, 128, HPC, 128), f32r, kind="ExternalInput")
    bqs_d = nc.dram_tensor("bqs", (D,), f32, kind="ExternalInput")
    bk_d = nc.dram_tensor("bk", (D,), f32, kind="ExternalInput")
    bvr_d = nc.dram_tensor("bvr", (1, D), f32r, kind="ExternalInput")
    bo_d = nc.dram_tensor("bo", (D,), f32, kind="ExternalInput")
    ones1_d = nc.dram_tensor("ones1", (1, 128), f32r, kind="ExternalInput")
    onescol_d = nc.dram_tensor("onescol", (128, 1), f32r, kind="ExternalInput")
    out_d = nc.dram_tensor("out", (D, S), f32, kind="ExternalOutput")

    with tile.TileContext(nc) as tc:
        with (
            tc.tile_pool(name="resident", bufs=1) as rpool,
            tc.tile_pool(name="dram", bufs=1, space="DRAM") as dpool,
        ):
            # long-lived SBUF
            kt_sb = rpool.tile([128, MC, RPC], f32r)      # R_k^T  [64KB/part]
            bq_sb = rpool.tile([128, MC], f32)
            bk_sb = rpool.tile([128, MC], f32)
            bo_sb = rpool.tile([128, MC], f32)
            bv_sb = rpool.tile([1, D], f32r)
            ones1 = rpool.tile([1, 128], f32r)
            onescol = rpool.tile([128, 1], f32r)
            nc.sync.dma_start(bq_sb[:], bqs_d.ap().rearrange("(t p) -> p t", p=128))
            nc.sync.dma_start(bk_sb[:], bk_d.ap().rearrange("(t p) -> p t", p=128))
            nc.sync.dma_start(bo_sb[:], bo_d.ap().rearrange("(t p) -> p t", p=128))
            nc.sync.dma_start(bv_sb[:], bvr_d.ap())
            nc.sync.dma_start(ones1[:], ones1_d.ap())
            nc.sync.dma_start(onescol[:], onescol_d.ap())

            # DRAM scratch
            qhat_dram = dpool.tile([MC, 128, RPC], f32r)  # [ct][dk][r]
            vhat_dram = dpool.tile([RPC, D], f32r)        # natural R_v

            # ---------------- phase V: R_v = VTs^T @ WvT + bv -------------
            with (
                nc.named_scope("proj_v"),
                tc.tile_pool(name="vin", bufs=1) as vin_pool,
                tc.tile_pool(name="wv", bufs=2) as wv_pool,
                tc.tile_pool(name="vps", bufs=4, space="PSUM") as vps_pool,
                tc.tile_pool(name="vout", bufs=4) as vout_pool,
            ):
                vt_st = vin_pool.tile([128, MC, RPC], f32r)
                for mc in range(MC):
                    nc.sync.dma_start(vt_st[:, mc, :], vts_d.ap()[mc])
                for cb in range(8):           # c blocks of 256
                    wv_st = wv_pool.tile([128, MC, 256], f32r, tag="wv")
                    nc.sync.dma_start(wv_st[:], wvt_d.ap()[cb])
                    for rt in range(8):       # r tiles of 128
                        ps = vps_pool.tile([128, 256], f32, tag="vps")
                        for mc in range(MC):
                            nc.tensor.matmul(
                                ps[:], vt_st[:, mc, 128 * rt:128 * rt + 128],
                                wv_st[:, mc, :], start=(mc == 0), stop=False)
                        nc.tensor.matmul(
                            ps[:], ones1[:],
                            bv_sb[:, 256 * cb:256 * cb + 256],
                            start=False, stop=True)
                        vo = vout_pool.tile([128, 256], f32r, tag="vo")
                        nc.vector.tensor_copy(vo[:], ps[:])
                        nc.gpsimd.dma_start(
                            vhat_dram[128 * rt:128 * rt + 128,
                                      256 * cb:256 * cb + 256], vo[:])

            # ---------------- phase Q: R_q^T = WqT^T @ QTs (scaled) -------
            with (
                nc.named_scope("proj_q"),
                tc.tile_pool(name="qin", bufs=1) as qin_pool,
                tc.tile_pool(name="wq", bufs=3) as wq_pool,
                tc.tile_pool(name="qps", bufs=4, space="PSUM") as qps_pool,
                tc.tile_pool(name="qout", bufs=4) as qout_pool,
            ):
                qt_st = qin_pool.tile([128, MC, RPC], f32r)
                for mc in range(MC):
                    nc.sync.dma_start(qt_st[:, mc, :], qts_d.ap()[mc])
                for ct in range(MC):
                    wq_st = wq_pool.tile([128, MC, 128], f32r, tag="wq")
                    nc.sync.dma_start(wq_st[:], wqt_d.ap()[ct])
                    for rb in range(2):
                        ps = qps_pool.tile([128, 512], f32, tag="qps")
                        for mc in range(MC):
                            nc.tensor.matmul(
                                ps[:], wq_st[:, mc, :],
                                qt_st[:, mc, 512 * rb:512 * rb + 512],
                                start=(mc == 0), stop=(mc == MC - 1))
                        qo = qout_pool.tile([128, 512], f32r, tag="qo")
                        nc.scalar.activation(qo[:], ps[:], AF.Identity,
                                             bias=bq_sb[:, ct:ct + 1],
                                             scale=SCALE)
                        nc.gpsimd.dma_start(
                            qhat_dram[ct, :, 512 * rb:512 * rb + 512], qo[:])

            # ---------------- phase K: R_k^T -> SBUF ----------------------
            with (
                nc.named_scope("proj_k"),
                tc.tile_pool(name="kin", bufs=1) as kin_pool,
                tc.tile_pool(name="wk", bufs=3) as wk_pool,
                tc.tile_pool(name="kps", bufs=4, space="PSUM") as kps_pool,
            ):
                kt_st = kin_pool.tile([128, MC, RPC], f32r)
                for mc in range(MC):
                    nc.sync.dma_start(kt_st[:, mc, :], kts_d.ap()[mc])
                for ct in range(MC):
                    wk_st = wk_pool.tile([128, MC, 128], f32r, tag="wk")
                    nc.sync.dma_start(wk_st[:], wkt_d.ap()[ct])
                    for rb in range(2):
                        ps = kps_pool.tile([128, 512], f32, tag="kps")
                        for mc in range(MC):
                            nc.tensor.matmul(
                                ps[:], wk_st[:, mc, :],
                                kt_st[:, mc, 512 * rb:512 * rb + 512],
                                start=(mc == 0), stop=(mc == MC - 1))
                        nc.scalar.activation(
                            kt_sb[:, ct, 512 * rb:512 * rb + 512], ps[:],
                            AF.Identity, bias=bk_sb[:, ct:ct + 1], scale=1.0)

            # ---------------- attention + output projection ---------------
            with (
                nc.named_scope("attn"),
                tc.tile_pool(name="qrhs", bufs=3) as q_pool,
                tc.tile_pool(name="vh", bufs=2) as vh_pool,
                tc.tile_pool(name="expp", bufs=6) as exp_pool,
                tc.tile_pool(name="tree", bufs=2) as tree_pool,
                tc.tile_pool(name="scps", bufs=2, space="PSUM") as scps_pool,
                tc.tile_pool(name="xps", bufs=2, space="PSUM") as xps_pool,
                tc.tile_pool(name="sps", bufs=1, space="PSUM") as sps_pool,
                tc.tile_pool(name="ops", bufs=1, space="PSUM") as ops_pool,
                tc.tile_pool(name="nrm", bufs=2) as nrm_pool,
                tc.tile_pool(name="xsb", bufs=2) as x_pool,
                tc.tile_pool(name="wo", bufs=3) as wo_pool,
                tc.tile_pool(name="oout", bufs=4) as oout_pool,
            ):
                for j in range(4):            # query pi-blocks of 512
                    x_j = x_pool.tile([128, HPC, 512], f32r, tag="xj")
                    for h in range(HPC):
                        q_rhs = q_pool.tile([128, 4, 128], f32r, tag="qr")
                        for t in range(4):
                            nc.sync.dma_start(
                                q_rhs[:, t, :],
                                qhat_dram[4 * j + t, :,
                                          128 * h:128 * h + 128])
                        v_h = vh_pool.tile([128, MC, 128], f32r, tag="vh")
                        nc.sync.dma_start(
                            v_h[:], vhat_dram[128 * h:128 * h + 128, :])
                        x_ps = xps_pool.tile([128, 512], f32, tag="xps")
                        s_ps = sps_pool.tile([1, 512], f32, tag="sps")
                        acc = tree_pool.tile([128, 512], f32, tag="t1")
                        tsum = tree_pool.tile([128, 512], f32r, tag="tf")
                        ex_halves = []
                        for tp in range(MC // 2):   # key-tile pairs
                            sc = scps_pool.tile([128, 2, 512], f32, tag="sc")
                            for i in range(2):
                                tk = 2 * tp + i
                                nc.tensor.matmul(
                                    sc[:, i, :],
                                    kt_sb[:, tk, 128 * h:128 * h + 128],
                                    q_rhs[:, :, :], start=True, stop=True)
                            ex = exp_pool.tile([128, 2, 512], f32r, tag="ex")
                            nc.scalar.activation(ex[:], sc[:], AF.Exp,
                                                 scale=1.0)
                            for i in range(2):
                                nc.tensor.matmul(
                                    x_ps[:], v_h[:, 2 * tp + i, :],
                                    ex[:, i, :], start=(tp == 0 and i == 0),
                                    stop=(tp == MC // 2 - 1 and i == 1))
                            # sequential softmax-denominator accumulation
                            e0 = ex[:, 0, :].bitcast(f32)
                            e1 = ex[:, 1, :].bitcast(f32)
                            if tp == 0:
                                nc.vector.tensor_add(acc[:], e0, e1)
                            elif tp < MC // 2 - 1:
                                nc.vector.tensor_add(acc[:], acc[:], e0)
                                nc.vector.tensor_add(acc[:], acc[:], e1)
                            else:
                                nc.vector.tensor_add(acc[:], acc[:], e0)
                                nc.vector.tensor_add(tsum[:], acc[:], e1)
                        nc.tensor.matmul(s_ps[:], onescol[:], tsum[:],
                                         start=True, stop=True)
                        rec = nrm_pool.tile([1, 512], f32, tag="rec")
                        nc.vector.reciprocal_approx_fast(rec[:], s_ps[:])
                        bcast = nrm_pool.tile([128, 512], f32, tag="bc")
                        nc.gpsimd.partition_broadcast(bcast[:], rec[:])
                        nc.vector.tensor_mul(x_j[:, h, :], x_ps[:], bcast[:])
                    # output projection for this pi-block
                    for ot in range(MC):
                        wo_st = wo_pool.tile([128, HPC, 128], f32r, tag="wo")
                        nc.sync.dma_start(wo_st[:], wot_d.ap()[ot])
                        op = ops_pool.tile([128, 512], f32, tag="op")
                        for h in range(HPC):
                            nc.tensor.matmul(op[:], wo_st[:, h, :],
                                             x_j[:, h, :], start=(h == 0),
                                             stop=(h == HPC - 1))
                        oo = oout_pool.tile([128, 512], f32, tag="oo")
                        nc.scalar.activation(oo[:], op[:], AF.Identity,
                                             bias=bo_sb[:, ot:ot + 1],
                                             scale=1.0)
                        nc.gpsimd.dma_start(
                            out_d.ap()[128 * ot:128 * ot + 128,
                                       512 * j:512 * j + 512], oo[:])

    nc.compile()
    return nc


def _prep_shared(Wq, Wk, Wv, Wo, bq, bk, bv, bo):
    wqt = _round_f32r(np.ascontiguousarray(np.asarray(Wq, np.float32).T))
    wkt = _round_f32r(np.ascontiguousarray(np.asarray(Wk, np.float32).T))
    wvt = _round_f32r(np.ascontiguousarray(np.asarray(Wv, np.float32).T))
    # wq/wk tiled: [ct][p=m%128][mc][c]
    wqt_t = np.ascontiguousarray(
        wqt.reshape(MC, 128, MC, 128).transpose(2, 1, 0, 3))
    wkt_t = np.ascontiguousarray(
        wkt.reshape(MC, 128, MC, 128).transpose(2, 1, 0, 3))
    # wv tiled: [cb][p][mc][c256]
    wvt_t = np.ascontiguousarray(
        wvt.reshape(MC, 128, 8, 256).transpose(2, 1, 0, 3))
    woT = np.ascontiguousarray(np.asarray(Wo, np.float32).T)
    bqs = (np.asarray(bq, np.float32) * SCALE).copy()
    bk_np = np.asarray(bk, np.float32).copy()
    bvr = _round_f32r(np.asarray(bv, np.float32).reshape(1, D))
    bo_np = np.asarray(bo, np.float32).copy()
    return wqt_t, wkt_t, wvt_t, woT, bqs, bk_np, bvr, bo_np


def kernel(Q, K, V, Wq, bq, Wk, bk, Wv, bv, Wo, bo, num_heads):
    global last_results
    assert int(num_heads) == H

    from concourse.bass_utils import run_bass_kernel_spmd

    if "nc" not in _cache:
        _cache["nc"] = _build()
    nc = _cache["nc"]

    Q = np.asarray(Q, np.float32)
    K = np.asarray(K, np.float32)
    V = np.asarray(V, np.float32)
    wqt_t, wkt_t, wvt_t, woT, bqs, bk_np, bvr, bo_np = _prep_shared(
        Wq, Wk, Wv, Wo, bq, bk, bv, bo)
    ones1 = np.ones((1, 128), np.float32)
    onescol = np.ones((128, 1), np.float32)

    in_maps = []
    for c in range(NC_):
        b, half = divmod(c, 2)
        r0 = RPC * half
        # wot tiled: [ot][dk][h][o] from WoT rows r0..r0+1024
        wot_t = np.ascontiguousarray(
            _round_f32r(woT[r0:r0 + RPC, :])
            .reshape(HPC, 128, MC, 128).transpose(2, 1, 0, 3))
        in_maps.append({
            "qts": _round_f32r(Q[b].T[:, r0:r0 + RPC]).reshape(MC, 128, RPC),
            "kts": _round_f32r(K[b].T[:, r0:r0 + RPC]).reshape(MC, 128, RPC),
            "vts": _round_f32r(V[b].T[:, r0:r0 + RPC]).reshape(MC, 128, RPC),
            "wqt": wqt_t, "wkt": wkt_t, "wvt": wvt_t, "wot": wot_t,
            "bqs": bqs, "bk": bk_np, "bvr": bvr, "bo": bo_np,
            "ones1": ones1, "onescol": onescol,
        })

    res = run_bass_kernel_spmd(nc, in_maps, core_ids=list(range(NC_)))
    last_results = res

    out = np.empty((B, S, D), np.float32)
    for b in range(B):
        oT = res.results[2 * b]["out"] + res.results[2 * b + 1]["out"]
        # oT[o, pi], pi = 128*t + u ; s = 16*u + t
        out[b] = oT.reshape(D, 16, 128).transpose(2, 1, 0).reshape(S, D)
    return out


# revision 9
# speedup vs baseline: 1.4797x; 1.1055x over previous
"""MultiHeadAttention (B=4, S=2048, D=2048, H=16) on 8 TRN2 NeuronCores.

Sharding: core c handles batch b = c//2 and head-half = c%2 (8 heads).
Each core computes Q/K/V projections for its 1024 rows, attention for its
8 heads, and a partial output projection; the host sums the two partials
per batch and un-permutes.

Layout trick: torch's `view(B, H, S, dk)` head split (no transpose) means
head h of batch b lives in rows [128h, 128h+128) of the projection output,
with each row holding 16 consecutive seq positions. Working in permuted
query/key coordinates pi = 128*t + u (s = 16*u + t), every attention
operand is an exact 128x128 tile of either the transposed projection
(R^T, for Q/K) or the natural projection (R, for V). Softmax is
permutation-invariant, and the host un-permutes the final output.

All matmuls run in float32r (fp32 with 10-bit mantissa, full PE speed);
host pre-rounds all external matmul operands. Weights are pre-tiled on
the host so every SBUF stage tile loads as one DMA with >=4KB contiguous
runs per partition.
"""
import math
import os

import numpy as np

B, S, D, H = 4, 2048, 2048, 16
DK = D // H            # 128
HPC = H // 2           # heads per core = 8
RPC = HPC * DK         # rows per core = 1024
NC_ = 8                # cores
MC = D // 128          # contraction chunks = 16
SCALE = 1.0 / math.sqrt(DK)

_cache = {}
last_results = None


def _round_f32r(x):
    """Round fp32 to the 10-bit-mantissa grid the PE uses for float32r."""
    x = np.ascontiguousarray(x, dtype=np.float32)
    u = x.view(np.uint32)
    lsb = (u >> np.uint32(13)) & np.uint32(1)
    r = (u + np.uint32(0x0FFF) + lsb) & np.uint32(0xFFFFE000)
    return r.view(np.float32)


def _build():
    import concourse.bass as bass
    import concourse.mybir as mybir
    import concourse.tile as tile
    from concourse import bacc

    f32 = mybir.dt.float32
    f32r = mybir.dt.float32r
    AF = mybir.ActivationFunctionType

    nc = bacc.Bacc("TRN2", target_bir_lowering=False, debug=False,
                   num_devices=NC_)

    # ---- external I/O ----
    # transposed input slices (m-chunked): [mc][p][r]
    qts_d = nc.dram_tensor("qts", (MC, 128, RPC), f32r, kind="ExternalInput")
    kts_d = nc.dram_tensor("kts", (MC, 128, RPC), f32r, kind="ExternalInput")
    vts_d = nc.dram_tensor("vts", (MC, 128, RPC), f32r, kind="ExternalInput")
    # pre-tiled weights: wq/wk [ct][p][mc][c], wv [cb][p][mc][c256]
    wqt_d = nc.dram_tensor("wqt", (MC, 128, MC, 128), f32r, kind="ExternalInput")
    wkt_d = nc.dram_tensor("wkt", (MC, 128, MC, 128), f32r, kind="ExternalInput")
    wvt_d = nc.dram_tensor("wvt", (8, 128, MC, 256), f32r, kind="ExternalInput")
    # pre-tiled output weights: [ot][dk][h][o]
    wot_d = nc.dram_tensor("wot", (MC, 128, HPC, 128), f32r, kind="ExternalInput")
    bqs_d = nc.dram_tensor("bqs", (D,), f32, kind="ExternalInput")
    bk_d = nc.dram_tensor("bk", (D,), f32, kind="ExternalInput")
    bvr_d = nc.dram_tensor("bvr", (1, D), f32r, kind="ExternalInput")
    bo_d = nc.dram_tensor("bo", (D,), f32, kind="ExternalInput")
    ones1_d = nc.dram_tensor("ones1", (1, 128), f32r, kind="ExternalInput")
    onescol_d = nc.dram_tensor("onescol", (128, 1), f32r, kind="ExternalInput")
    out_d = nc.dram_tensor("out", (D, S), f32, kind="ExternalOutput")

    with tile.TileContext(nc) as tc:
        with (
            tc.tile_pool(name="resident", bufs=1) as rpool,
            tc.tile_pool(name="dram", bufs=1, space="DRAM") as dpool,
        ):
            # long-lived SBUF
            kt_sb = rpool.tile([128, MC, RPC], f32r)      # R_k^T  [64KB/part]
            bq_sb = rpool.tile([128, MC], f32)
            bk_sb = rpool.tile([128, MC], f32)
            bo_sb = rpool.tile([128, MC], f32)
            bv_sb = rpool.tile([1, D], f32r)
            ones1 = rpool.tile([1, 128], f32r)
            onescol = rpool.tile([128, 1], f32r)
            nc.sync.dma_start(bq_sb[:], bqs_d.ap().rearrange("(t p) -> p t", p=128))
            nc.sync.dma_start(bk_sb[:], bk_d.ap().rearrange("(t p) -> p t", p=128))
            nc.sync.dma_start(bo_sb[:], bo_d.ap().rearrange("(t p) -> p t", p=128))
            nc.sync.dma_start(bv_sb[:], bvr_d.ap())
            nc.sync.dma_start(ones1[:], ones1_d.ap())
            nc.sync.dma_start(onescol[:], onescol_d.ap())

            # DRAM scratch
            qhat_dram = dpool.tile([MC, 128, RPC], f32r)  # [ct][dk][r]
            vhat_dram = dpool.tile([RPC, D], f32r)        # natural R_v

            # ---------------- phase V: R_v = VTs^T @ WvT + bv -------------
            with (
                nc.named_scope("proj_v"),
                tc.tile_pool(name="vin", bufs=1) as vin_pool,
                tc.tile_pool(name="wv", bufs=2) as wv_pool,
                tc.tile_pool(name="vps", bufs=4, space="PSUM") as vps_pool,
                tc.tile_pool(name="vout", bufs=4) as vout_pool,
            ):
                vt_st = vin_pool.tile([128, MC, RPC], f32r)
                for mc in range(MC):
                    nc.sync.dma_start(vt_st[:, mc, :], vts_d.ap()[mc])
                for cb in range(8):           # c blocks of 256
                    wv_st = wv_pool.tile([128, MC, 256], f32r, tag="wv")
                    nc.sync.dma_start(wv_st[:], wvt_d.ap()[cb])
                    for rt in range(8):       # r tiles of 128
                        ps = vps_pool.tile([128, 256], f32, tag="vps")
                        for mc in range(MC):
                            nc.tensor.matmul(
                                ps[:], vt_st[:, mc, 128 * rt:128 * rt + 128],
                                wv_st[:, mc, :], start=(mc == 0), stop=False)
                        nc.tensor.matmul(
                            ps[:], ones1[:],
                            bv_sb[:, 256 * cb:256 * cb + 256],
                            start=False, stop=True)
                        vo = vout_pool.tile([128, 256], f32r, tag="vo")
                        nc.vector.tensor_copy(vo[:], ps[:])
                        nc.gpsimd.dma_start(
                            vhat_dram[128 * rt:128 * rt + 128,
                                      256 * cb:256 * cb + 256], vo[:])

            # ---------------- phase Q: R_q^T = WqT^T @ QTs (scaled) -------
            with (
                nc.named_scope("proj_q"),
                tc.tile_pool(name="qin", bufs=1) as qin_pool,
                tc.tile_pool(name="wq", bufs=3) as wq_pool,
                tc.tile_pool(name="qps", bufs=4, space="PSUM") as qps_pool,
                tc.tile_pool(name="qout", bufs=4) as qout_pool,
            ):
                qt_st = qin_pool.tile([128, MC, RPC], f32r)
                for mc in range(MC):
                    nc.sync.dma_start(qt_st[:, mc, :], qts_d.ap()[mc])
                for ct in range(MC):
                    wq_st = wq_pool.tile([128, MC, 128], f32r, tag="wq")
                    nc.sync.dma_start(wq_st[:], wqt_d.ap()[ct])
                    for rb in range(2):
                        ps = qps_pool.tile([128, 512], f32, tag="qps")
                        for mc in range(MC):
                            nc.tensor.matmul(
                                ps[:], wq_st[:, mc, :],
                                qt_st[:, mc, 512 * rb:512 * rb + 512],
                                start=(mc == 0), stop=(mc == MC - 1))
                        qo = qout_pool.tile([128, 512], f32r, tag="qo")
                        nc.scalar.activation(qo[:], ps[:], AF.Identity,
                                             bias=bq_sb[:, ct:ct + 1],
                                             scale=SCALE)
                        nc.gpsimd.dma_start(
                            qhat_dram[ct, :, 512 * rb:512 * rb + 512], qo[:])

            # ---------------- phase K: R_k^T -> SBUF ----------------------
            with (
                nc.named_scope("proj_k"),
                tc.tile_pool(name="kin", bufs=1) as kin_pool,
                tc.tile_pool(name="wk", bufs=3) as wk_pool,
                tc.tile_pool(name="kps", bufs=4, space="PSUM") as kps_pool,
            ):
                kt_st = kin_pool.tile([128, MC, RPC], f32r)
                for mc in range(MC):
                    nc.sync.dma_start(kt_st[:, mc, :], kts_d.ap()[mc])
                for ct in range(MC):
                    wk_st = wk_pool.tile([128, MC, 128], f32r, tag="wk")
                    nc.sync.dma_start(wk_st[:], wkt_d.ap()[ct])
                    for rb in range(2):
                        ps = kps_pool.tile([128, 512], f32, tag="kps")
                        for mc in range(MC):
                            nc.tensor.matmul(
                                ps[:], wk_st[:, mc, :],
                                kt_st[:, mc, 512 * rb:512 * rb + 512],
                                start=(mc == 0), stop=(mc == MC - 1))
                        nc.scalar.activation(
                            kt_sb[:, ct, 512 * rb:512 * rb + 512], ps[:],
                            AF.Identity, bias=bk_sb[:, ct:ct + 1], scale=1.0)

            # ---------------- attention + output projection ---------------
            with (
                nc.named_scope("attn"),
                tc.tile_pool(name="qrhs", bufs=3) as q_pool,
                tc.tile_pool(name="vh", bufs=2) as vh_pool,
                tc.tile_pool(name="expp", bufs=6) as exp_pool,
                tc.tile_pool(name="tree", bufs=2) as tree_pool,
                tc.tile_pool(name="scps", bufs=2, space="PSUM") as scps_pool,
                tc.tile_pool(name="xps", bufs=2, space="PSUM") as xps_pool,
                tc.tile_pool(name="sps", bufs=1, space="PSUM") as sps_pool,
                tc.tile_pool(name="ops", bufs=1, space="PSUM") as ops_pool,
                tc.tile_pool(name="nrm", bufs=2) as nrm_pool,
                tc.tile_pool(name="xsb", bufs=2) as x_pool,
                tc.tile_pool(name="wo", bufs=3) as wo_pool,
                tc.tile_pool(name="oout", bufs=4) as oout_pool,
            ):
                NP_ = MC // 2     # key-tile pairs per head

                def emit_outproj(j, x_j):
                    for ot in range(MC):
                        wo_st = wo_pool.tile([128, HPC, 128], f32r, tag="wo")
                        nc.sync.dma_start(wo_st[:], wot_d.ap()[ot])
                        op = ops_pool.tile([128, 512], f32, tag="op")
                        for h in range(HPC):
                            nc.tensor.matmul(op[:], wo_st[:, h, :],
                                             x_j[:, h, :], start=(h == 0),
                                             stop=(h == HPC - 1))
                        oo = oout_pool.tile([128, 512], f32, tag="oo")
                        nc.scalar.activation(oo[:], op[:], AF.Identity,
                                             bias=bo_sb[:, ot:ot + 1],
                                             scale=1.0)
                        nc.gpsimd.dma_start(
                            out_d.ap()[128 * ot:128 * ot + 128,
                                       512 * j:512 * j + 512], oo[:])

                prev = None   # (j, x_j) awaiting output projection
                for j in range(4):            # query pi-blocks of 512
                    x_j = x_pool.tile([128, HPC, 512], f32r, tag="xj")
                    for h in range(HPC):
                        q_rhs = q_pool.tile([128, 4, 128], f32r, tag="qr")
                        for t in range(4):
                            nc.sync.dma_start(
                                q_rhs[:, t, :],
                                qhat_dram[4 * j + t, :,
                                          128 * h:128 * h + 128])
                        v_h = vh_pool.tile([128, MC, 128], f32r, tag="vh")
                        nc.sync.dma_start(
                            v_h[:], vhat_dram[128 * h:128 * h + 128, :])
                        x_ps = xps_pool.tile([128, 512], f32, tag="xps")
                        s_ps = sps_pool.tile([1, 512], f32, tag="sps")
                        acc = tree_pool.tile([128, 512], f32, tag="acc")
                        tsum = tree_pool.tile([128, 512], f32r, tag="tf")

                        exs = [None] * NP_
                        t1s = [None] * NP_

                        def pv_and_sum(tp):
                            ex = exs[tp]
                            for i in range(2):
                                nc.tensor.matmul(
                                    x_ps[:], v_h[:, 2 * tp + i, :],
                                    ex[:, i, :], start=(tp == 0 and i == 0),
                                    stop=(tp == NP_ - 1 and i == 1))
                            t1 = tree_pool.tile([128, 512], f32, tag="t1")
                            nc.vector.tensor_add(
                                t1[:], ex[:, 0, :].bitcast(f32),
                                ex[:, 1, :].bitcast(f32))
                            t1s[tp] = t1
                            # fold into running accumulator
                            if tp == 1:
                                nc.vector.tensor_add(acc[:], t1s[0][:], t1[:])
                            elif 1 < tp < NP_ - 1:
                                nc.vector.tensor_add(acc[:], acc[:], t1[:])
                            elif tp == NP_ - 1:
                                nc.vector.tensor_add(tsum[:], acc[:], t1[:])

                        # software-pipelined: scores/exp run one pair ahead
                        for tp in range(NP_):
                            sc = scps_pool.tile([128, 2, 512], f32, tag="sc")
                            for i in range(2):
                                tk = 2 * tp + i
                                nc.tensor.matmul(
                                    sc[:, i, :],
                                    kt_sb[:, tk, 128 * h:128 * h + 128],
                                    q_rhs[:, :, :], start=True, stop=True)
                            ex = exp_pool.tile([128, 2, 512], f32r, tag="ex")
                            nc.scalar.activation(ex[:], sc[:], AF.Exp,
                                                 scale=1.0)
                            exs[tp] = ex
                            if tp >= 1:
                                pv_and_sum(tp - 1)
                        pv_and_sum(NP_ - 1)

                        nc.tensor.matmul(s_ps[:], onescol[:], tsum[:],
                                         start=True, stop=True)
                        rec = nrm_pool.tile([1, 512], f32, tag="rec")
                        nc.vector.reciprocal_approx_fast(rec[:], s_ps[:])
                        bcast = nrm_pool.tile([128, 512], f32, tag="bc")
                        nc.gpsimd.partition_broadcast(bcast[:], rec[:])
                        nc.vector.tensor_mul(x_j[:, h, :], x_ps[:], bcast[:])
                    if prev is not None:
                        emit_outproj(*prev)
                    prev = (j, x_j)
                emit_outproj(*prev)

    nc.compile()
    return nc


def _prep_shared(Wq, Wk, Wv, Wo, bq, bk, bv, bo):
    wqt = _round_f32r(np.ascontiguousarray(np.asarray(Wq, np.float32).T))
    wkt = _round_f32r(np.ascontiguousarray(np.asarray(Wk, np.float32).T))
    wvt = _round_f32r(np.ascontiguousarray(np.asarray(Wv, np.float32).T))
    # wq/wk tiled: [ct][p=m%128][mc][c]
    wqt_t = np.ascontiguousarray(
        wqt.reshape(MC, 128, MC, 128).transpose(2, 1, 0, 3))
    wkt_t = np.ascontiguousarray(
        wkt.reshape(MC, 128, MC, 128).transpose(2, 1, 0, 3))
    # wv tiled: [cb][p][mc][c256]
    wvt_t = np.ascontiguousarray(
        wvt.reshape(MC, 128, 8, 256).transpose(2, 1, 0, 3))
    woT = np.ascontiguousarray(np.asarray(Wo, np.float32).T)
    bqs = (np.asarray(bq, np.float32) * SCALE).copy()
    bk_np = np.asarray(bk, np.float32).copy()
    bvr = _round_f32r(np.asarray(bv, np.float32).reshape(1, D))
    bo_np = np.asarray(bo, np.float32).copy()
    return wqt_t, wkt_t, wvt_t, woT, bqs, bk_np, bvr, bo_np


def kernel(Q, K, V, Wq, bq, Wk, bk, Wv, bv, Wo, bo, num_heads):
    global last_results
    assert int(num_heads) == H

    from concourse.bass_utils import run_bass_kernel_spmd

    if "nc" not in _cache:
        _cache["nc"] = _build()
    nc = _cache["nc"]

    Q = np.asarray(Q, np.float32)
    K = np.asarray(K, np.float32)
    V = np.asarray(V, np.float32)
    wqt_t, wkt_t, wvt_t, woT, bqs, bk_np, bvr, bo_np = _prep_shared(
        Wq, Wk, Wv, Wo, bq, bk, bv, bo)
    ones1 = np.ones((1, 128), np.float32)
    onescol = np.ones((128, 1), np.float32)

    in_maps = []
    for c in range(NC_):
        b, half = divmod(c, 2)
        r0 = RPC * half
        # wot tiled: [ot][dk][h][o] from WoT rows r0..r0+1024
        wot_t = np.ascontiguousarray(
            _round_f32r(woT[r0:r0 + RPC, :])
            .reshape(HPC, 128, MC, 128).transpose(2, 1, 0, 3))
        in_maps.append({
            "qts": _round_f32r(Q[b].T[:, r0:r0 + RPC]).reshape(MC, 128, RPC),
            "kts": _round_f32r(K[b].T[:, r0:r0 + RPC]).reshape(MC, 128, RPC),
            "vts": _round_f32r(V[b].T[:, r0:r0 + RPC]).reshape(MC, 128, RPC),
            "wqt": wqt_t, "wkt": wkt_t, "wvt": wvt_t, "wot": wot_t,
            "bqs": bqs, "bk": bk_np, "bvr": bvr, "bo": bo_np,
            "ones1": ones1, "onescol": onescol,
        })

    res = run_bass_kernel_spmd(nc, in_maps, core_ids=list(range(NC_)))
    last_results = res

    out = np.empty((B, S, D), np.float32)
    for b in range(B):
        oT = res.results[2 * b]["out"] + res.results[2 * b + 1]["out"]
        # oT[o, pi], pi = 128*t + u ; s = 16*u + t
        out[b] = oT.reshape(D, 16, 128).transpose(2, 1, 0).reshape(S, D)
    return out
